# revision 48
# baseline (speedup 1.0000x reference)
"""Causal self-attention (GQA + RMS-norm + RoPE) Trainium2 Bass kernel.

Sharding over 8 NeuronCores: 2-way data parallel (batch) x 4-way head
parallel (one GQA group of 4 q-heads + 1 kv-head per core).  Each core
computes q/k/v projections for its group, flash-style causal attention
(scores kept transposed [k, q] so softmax sums ride the tensor engine),
and a partial output projection.  Host sums the 4 per-group partials per
batch.

All matmul operands are bf16 (fp32 accumulation in PSUM); softmax /
norm statistics are fp32.  RMS-norm bounds |scores| <= gain*sqrt(hd), so
softmax needs no max-subtraction.
"""

import numpy as np
import ml_dtypes

import concourse.bacc as bacc
import concourse.mybir as mybir
from concourse.tile import TileContext
from concourse.bass_utils import run_bass_kernel_spmd

BF16 = mybir.dt.bfloat16
F32 = mybir.dt.float32
F32R = mybir.dt.float32r
AF = mybir.ActivationFunctionType
bf = ml_dtypes.bfloat16

B, S, D = 2, 2048, 2048
H, HKV, HD = 16, 4, 128
RQ = H // HKV            # q heads per kv group (4)
NCORES = 8
NDT = D // 128           # 16 contraction tiles
NST = S // 512           # 4 query/sequence 512-tiles
NKT = S // 128           # 16 key 128-tiles
EPS = float(np.finfo(np.float32).eps)

_PROG_CACHE = {}


def _build_program(n_timing_iters=1, phases="full"):
    nc = bacc.Bacc("TRN2", debug=False, enable_asserts=False, num_devices=NCORES)

    xT_d = nc.dram_tensor("xT", [128, NDT, S], BF16, kind="ExternalInput")
    wqT_d = nc.dram_tensor("wqT", [128, NDT, RQ * HD], BF16, kind="ExternalInput")
    wkT_d = nc.dram_tensor("wkT", [128, NDT, HD], BF16, kind="ExternalInput")
    wvT_d = nc.dram_tensor("wvT", [128, NDT, HD], BF16, kind="ExternalInput")
    wpT_d = nc.dram_tensor("wpT", [NDT, 128, RQ * 128], BF16, kind="ExternalInput")
    cosF_d = nc.dram_tensor("cosF", [128, S], BF16, kind="ExternalInput")
    sinF_d = nc.dram_tensor("sinF", [128, S], BF16, kind="ExternalInput")
    cfs_d = nc.dram_tensor("cfs", [1, 641], F32R, kind="ExternalInput")
    onescol_f_d = nc.dram_tensor("onescol_f", [128, 1], F32R, kind="ExternalInput")
    onescol_b_d = nc.dram_tensor("onescol_b", [128, 1], BF16, kind="ExternalInput")
    idtr_d = nc.dram_tensor("idtr", [128, 256], BF16, kind="ExternalInput")
    outT_d = nc.dram_tensor("outT", [NDT, 128, S], BF16, kind="ExternalOutput")

    with TileContext(nc) as tc:
        with tc.tile_pool(name="res", bufs=1) as res, \
             tc.tile_pool(name="work", bufs=2) as wk, \
             tc.tile_pool(name="pwork", bufs=2, space="PSUM") as pw:

            # ---- resident tiles (allocated once) ----
            xT = res.tile([128, NDT, S], BF16)             # [d-part, dt, s]
            wqT = res.tile([128, NDT, RQ * HD], BF16)
            wkT = res.tile([128, NDT, HD], BF16)
            wvT = res.tile([128, NDT, HD], BF16)
            cosF = res.tile([128, S], BF16)
            sinF = res.tile([128, S], BF16)
            cfs = res.tile([1, 641], F32R)
            onescol_f = res.tile([128, 1], F32R)
            onescol_b = res.tile([128, 1], BF16)
            idtr = res.tile([128, 256], BF16)
            kT = res.tile([128, S], BF16)
            qT = [res.tile([128, S], BF16, name=f"qT{h}", tag=f"qT{h}")
                  for h in range(RQ)]
            yT = [res.tile([128, S], BF16, name=f"yT{h}", tag=f"yT{h}")
                  for h in range(RQ)]
            vTst = res.tile([128, S], BF16)                # v^T staging
            V_all = res.tile([128, S], BF16)               # v natural, kt-major

            eps_ap = cfs[0:1, 0:1].bitcast(F32)
            ones_row = cfs[0:1, 1:129]
            ident = idtr[:, 0:128]
            triu = idtr[:, 128:256]

            def body(_iv=None):
                # ---- batched resident loads ----
                nc.scalar.dma_start(wkT[:], wkT_d[:])
                nc.scalar.dma_start(wvT[:], wvT_d[:])
                for c in range(4):
                    nc.sync.dma_start(xT[:, 4 * c:4 * c + 4, :],
                                      xT_d[:, 4 * c:4 * c + 4, :])
                nc.scalar.dma_start(wqT[:], wqT_d[:])
                nc.scalar.dma_start(cosF[:], cosF_d[:])
                nc.scalar.dma_start(sinF[:], sinF_d[:])
                nc.scalar.dma_start(cfs[:], cfs_d[:])
                nc.scalar.dma_start(onescol_f[:], onescol_f_d[:])
                nc.scalar.dma_start(onescol_b[:], onescol_b_d[:])
                nc.scalar.dma_start(idtr[:], idtr_d[:])
                nc.scalar.add_instruction(
                    mybir.InstLoadActFuncSet(
                        name=nc.get_next_instruction_name(),
                        ins=[], outs=[], act_func_set_id=6))

                if phases == "load":
                    return

                def proj_accum(wt_all, col_off, j, tag="big", tbufs=3):
                    """psum [128,512] = sum_d W[d].T @ xT[d, s-slice]"""
                    acc = pw.tile([128, 512], F32, name="acc", tag=tag,
                                  bufs=tbufs)
                    for dt in range(NDT):
                        lhsT = wt_all[:, dt, col_off:col_off + 128]
                        rhs = xT[:, dt, 512 * j:512 * j + 512]
                        nc.tensor.matmul(acc[:], lhsT, rhs,
                                         start=(dt == 0), stop=(dt == NDT - 1))
                    return acc

                def norm_rope_chain(acc, grow_slice, dest, j):
                    """RMS-norm + RoPE + scale; writes dest[:, 512j:+512] bf16.

                    Stats ride ACT/DVE/Pool; the only PE op is the ms
                    ones-matmul, emitted here (callers pipeline the call one
                    chain behind the mm chains to keep PE dense)."""
                    stage = wk.tile([128, 512], F32, name="stage", tag="stage", bufs=3)
                    nc.vector.tensor_copy(stage[:], acc[:])
                    swap = wk.tile([128, 512], F32, name="swap", tag="swap", bufs=3)
                    nc.scalar.dma_start(swap[0:64, :], stage[64:128, :])
                    nc.scalar.dma_start(swap[64:128, :], stage[0:64, :])
                    sq = wk.tile([128, 512], F32R, name="sq", tag="sq", bufs=2)
                    nc.vector.tensor_mul(sq[:], stage[:], stage[:])
                    ms = pw.tile([1, 512], F32, name="ms", tag="bcast", bufs=1)
                    nc.tensor.matmul(ms[:], onescol_f[:],
                                     sq[:], start=True, stop=True)
                    # rsqrt via ln+exp: keeps every ACT op in the single
                    # natural_log_exp_and_others table set (no set thrash
                    # against attention's Exp).
                    srt = wk.tile([1, 512], F32, name="srt", tag="srt", bufs=2)
                    nc.scalar.activation(srt[:], ms[:], AF.Ln,
                                         bias=eps_ap, scale=1.0 / HD)
                    rr = wk.tile([1, 512], F32, name="rr", tag="rr", bufs=2)
                    nc.scalar.activation(rr[:], srt[:], AF.Exp, scale=-0.5)
                    if grow_slice is not None:
                        # scale by gain/sqrt(hd) (per-head scalar)
                        rg = wk.tile([1, 512], F32, name="rg", tag="rg", bufs=2)
                        nc.vector.tensor_scalar_mul(rg[:], rr[:], grow_slice)
                    else:
                        rg = rr
                    Rb = wk.tile([128, 512], F32, name="Rb", tag="Rb", bufs=2)
                    nc.gpsimd.partition_broadcast(Rb[:], rg[0:1, :])
                    sl = slice(512 * j, 512 * j + 512)
                    nc.vector.tensor_mul(stage[:], stage[:], cosF[:, sl])
                    nc.vector.tensor_mul(swap[:], swap[:], sinF[:, sl])
                    nc.vector.tensor_add(stage[:], stage[:], swap[:])
                    nc.vector.tensor_mul(dest[:, sl], stage[:], Rb[:])

                # ---- projections: mm-chains pipelined one ahead of stats ----
                proj_jobs = []   # (kind, h, j); kind: 0=k, 1=v, 2=q
                for j in range(NST):
                    proj_jobs.append(("k", 0, j))
                for j in range(NST):
                    proj_jobs.append(("v", 0, j))

                def emit_proj_tail(kind, h, j, acc):
                    if kind == "k":
                        norm_rope_chain(acc, None, kT, j)
                    elif kind == "q":
                        grow_h = cfs[0:1, 129 + h:130 + h].bitcast(F32)
                        norm_rope_chain(acc, grow_h, qT[h], j)
                    else:
                        nc.vector.tensor_copy(vTst[:, 512 * j:512 * j + 512], acc[:])

                def run_proj_jobs(jobs, pending, tags=None, lag=1):
                    for idx, (kind, h, j) in enumerate(jobs):
                        tag, tbufs = ("big", 2) if tags is None else tags[idx]
                        if kind == "k":
                            acc = proj_accum(wkT, 0, j, tag, tbufs)
                        elif kind == "v":
                            acc = proj_accum(wvT, 0, j, tag, tbufs)
                        else:
                            acc = proj_accum(wqT, 128 * h, j, tag, tbufs)
                        pending.append((kind, h, j, acc))
                        if len(pending) > lag:
                            emit_proj_tail(*pending.pop(0))
                    return pending

                # first wave: 7-deep psum concurrency so the x-load DMA
                # wavefront keeps PE fed (every arriving x tile unlocks 7 mms)
                # 5-slot rotation (big 4KB x2 / acc 2KB x2 / small 2KB x1);
                # ms keeps its own bcast bank so chain tails never cycle
                rot5 = [("big", 2), ("big", 2), ("acc", 2), ("acc", 2),
                        ("small", 1)]
                kv_tags = [rot5[i % 5] for i in range(8)]
                pending = run_proj_jobs(proj_jobs, [], tags=kv_tags, lag=4)

                # ---- v^T -> V transposes (PE) ----
                while pending:
                    emit_proj_tail(*pending.pop(0))
                for kt in range(NKT):
                    tp = pw.tile([128, 128], BF16, name="tp", tag="acc", bufs=2)
                    nc.tensor.transpose(tp[:], vTst[:, 128 * kt:128 * kt + 128],
                                        ident)
                    nc.scalar.copy(V_all[:, 128 * kt:128 * kt + 128], tp[:])

                if phases == "kv":
                    return

                def attention_block(h, j):
                    """Causal attention for queries [512j, 512j+512), head h.
                    Consecutive k-tiles pair into one 4KB psum slot so a
                    single exp instruction covers both (ACT instruction rate
                    paces this phase); d/PV matmuls stay per-tile and trail
                    by one pair."""
                    nkt = 4 * j + 4
                    acc_y = pw.tile([128, 512], F32, name="acc_y", tag="acc",
                                    bufs=2)
                    acc_d = pw.tile([1, 512], F32, name="acc_d", tag="small",
                                    bufs=1)
                    lagged = []

                    def consume(kts, P2, c0s):
                        for i, kt in enumerate(kts):
                            c0 = c0s[i]
                            nc.tensor.matmul(acc_d[0:1, c0:512], onescol_b[:],
                                             P2[:, i, c0:512],
                                             start=(kt == 0),
                                             stop=(kt == nkt - 1),
                                             skip_group_check=True)
                            nc.tensor.matmul(acc_y[:, c0:512],
                                             V_all[:, 128 * kt:128 * kt + 128],
                                             P2[:, i, c0:512],
                                             start=(kt == 0),
                                             stop=(kt == nkt - 1),
                                             skip_group_check=True)

                    for ktp in range(nkt // 2):
                        kts = (2 * ktp, 2 * ktp + 1)
                        c0s = [max(0, 128 * (kt - 4 * j)) for kt in kts]
                        ps2 = pw.tile([128, 2, 512], F32, name="ps2",
                                      tag="big", bufs=2)
                        P2 = wk.tile([128, 2, 512], BF16, name="P2", tag="P",
                                     bufs=4)
                        for i, kt in enumerate(kts):
                            nc.tensor.matmul(
                                ps2[:, i, c0s[i]:512],
                                kT[:, 128 * kt:128 * kt + 128],
                                qT[h][:, 512 * j + c0s[i]:512 * j + 512],
                                start=True, stop=True)
                        # exp of the [c0s[0]:c0s[1]) sliver of slot 1 is
                        # stale psum (finite); its P2 region is never read
                        nc.scalar.activation(P2[:, :, c0s[0]:512],
                                             ps2[:, :, c0s[0]:512], AF.Exp)
                        for i, kt in enumerate(kts):
                            if kt >= 4 * j:
                                nc.vector.tensor_mul(
                                    P2[:, i, c0s[i]:c0s[i] + 128],
                                    P2[:, i, c0s[i]:c0s[i] + 128], triu)
                        lagged.append((kts, P2, c0s))
                        if len(lagged) > 1:
                            consume(*lagged.pop(0))
                    while lagged:
                        consume(*lagged.pop(0))
                    rd = wk.tile([1, 512], F32, name="rd", tag="rd", bufs=2)
                    nc.vector.reciprocal(rd[:], acc_d[:])
                    Rd = wk.tile([128, 512], F32, name="Rd", tag="Rb", bufs=2)
                    nc.gpsimd.partition_broadcast(Rd[:], rd[0:1, :])
                    nc.vector.tensor_mul(yT[h][:, 512 * j:512 * j + 512],
                                         acc_y[:], Rd[:])

                # ---- all q projections up front; attention then owns
                # the big psum slots exclusively ----
                q_jobs = [("q", h, j) for h in range(RQ) for j in range(NST)]
                q_tags = [rot5[i % 5] for i in range(len(q_jobs))]
                pending = run_proj_jobs(q_jobs, pending, tags=q_tags, lag=4)
                while pending:
                    emit_proj_tail(*pending.pop(0))
                if phases != "kvq":
                    for h in range(RQ):
                        for j in range(NST):
                            attention_block(h, j)

                # ---- output projection (transposed: out^T[D, s]) ----
                if phases in ("kv", "kvq", "noout"):
                    return
                ptags = ["big", "acc", "big", "acc"]
                pbufs = {"big": 2, "acc": 2}
                for dt in range(NDT):
                    wp = wk.tile([128, RQ * 128], BF16, name="wp", tag="wp",
                                 bufs=3)
                    nc.scalar.dma_start(wp[:], wpT_d[dt])
                    osb = wk.tile([128, S], BF16, name="osb", tag="osb")
                    for sjj in range(NST):
                        po = pw.tile([128, 512], F32, name=f"po{sjj}",
                                     tag=ptags[sjj], bufs=pbufs[ptags[sjj]])
                        for h in range(RQ):
                            nc.tensor.matmul(
                                po[:], wp[:, 128 * h:128 * h + 128],
                                yT[h][:, 512 * sjj:512 * sjj + 512],
                                start=(h == 0), stop=(h == RQ - 1))
                        nc.vector.tensor_copy(osb[:, 512 * sjj:512 * sjj + 512],
                                              po[:])
                    eng = nc.sync if dt % 2 == 0 else nc.scalar
                    eng.dma_start(outT_d[dt], osb[:])

            if n_timing_iters > 1:
                with tc.For_i(0, n_timing_iters, 1):
                    body()
            else:
                body()

    nc.compile()
    return nc


def _get_program(n_timing_iters=1, phases="full"):
    key = (n_timing_iters, phases)
    if key not in _PROG_CACHE:
        _PROG_CACHE[key] = _build_program(n_timing_iters, phases)
    return _PROG_CACHE[key]


def _host_inputs(x, Wq, Wk, Wv, Wproj, q_gain):
    """Build the 8 per-core input maps (host-side layout prep)."""
    half = HD // 2
    inv = 1.0 / (10000.0 ** (np.arange(0, HD, 2, dtype=np.float64) / HD))
    t = np.arange(S, dtype=np.float64)
    fr = np.outer(t, inv).astype(np.float32)          # [S, 64]
    cos = np.cos(fr).astype(np.float32)
    sin = np.sin(fr).astype(np.float32)
    cosF = np.concatenate([cos.T, cos.T], 0).astype(bf)          # [128, S]
    sinF = np.concatenate([sin.T, -sin.T], 0).astype(bf)

    onescol_f = np.ones((128, 1), np.float32)
    onescol_b = np.ones((128, 1), bf)
    ident = np.eye(128, dtype=np.float32)
    triu = (np.arange(128)[None, :] >= np.arange(128)[:, None]).astype(np.float32)
    idtr = np.concatenate([ident, triu], 1).astype(bf)

    def pmajor(wT, width):
        return np.ascontiguousarray(
            wT.reshape(NDT, 128, width).transpose(1, 0, 2))

    xT = [pmajor(np.ascontiguousarray(x[b].T).astype(bf), S) for b in range(B)]

    in_maps = []
    for c in range(NCORES):
        b, g = divmod(c, HKV) if NCORES == B * HKV else (None, None)
        # core layout: c = b * 4 + g
        b, g = c // HKV, c % HKV
        wq = np.ascontiguousarray(Wq[512 * g:512 * (g + 1)].T).astype(bf)
        wk_ = np.ascontiguousarray(Wk[128 * g:128 * (g + 1)].T).astype(bf)
        wv = np.ascontiguousarray(Wv[128 * g:128 * (g + 1)].T).astype(bf)
        wpT = np.ascontiguousarray(Wproj[:, 512 * g:512 * (g + 1)].T)  # [512, 2048]
        # [dt][c-part 128, (h, m) 512]: wpT2[dt, c, 128h+m] = Wp[128dt+m, 512g+128h+c]
        wpT = np.ascontiguousarray(
            wpT.reshape(RQ, 128, NDT, 128).transpose(2, 1, 0, 3).reshape(
                NDT, 128, RQ * 128)).astype(bf)
        cfsv = np.zeros((1, 641), np.float32)
        cfsv[0, 0] = EPS
        cfsv[0, 1:129] = 1.0
        gv = (q_gain[RQ * g: RQ * (g + 1)].astype(np.float64)
              / np.sqrt(HD)).astype(np.float32)
        cfsv[0, 129:133] = gv
        in_maps.append({
            "xT": xT[b],
            "wqT": pmajor(wq, RQ * HD),
            "wkT": pmajor(wk_, HD),
            "wvT": pmajor(wv, HD),
            "wpT": wpT,
            "cosF": cosF, "sinF": sinF, "cfs": cfsv,
            "onescol_f": onescol_f, "onescol_b": onescol_b, "idtr": idtr,
        })
    return in_maps


def kernel(x, Wq, Wk, Wv, Wproj, q_gain, _n_timing_iters=1, _return_raw=False):
    x = np.asarray(x, np.float32)
    in_maps = _host_inputs(np.asarray(x, np.float32),
                           np.asarray(Wq, np.float32),
                           np.asarray(Wk, np.float32),
                           np.asarray(Wv, np.float32),
                           np.asarray(Wproj, np.float32),
                           np.asarray(q_gain, np.float32))
    nc = _get_program(_n_timing_iters)
    res = run_bass_kernel_spmd(nc, in_maps, core_ids=list(range(NCORES)),
                               trace=False)
    if _return_raw:
        return res
    out = np.zeros((B, S, D), np.float32)
    for c in range(NCORES):
        b = c // HKV
        outT = res.results[c]["outT"].reshape(D, S).astype(np.float32)
        out[b] += outT.T
    return out


if __name__ == "__main__":
    rng = np.random.default_rng(0)
    x = rng.standard_normal((B, S, D)).astype(np.float32)
    Wq = (rng.standard_normal((D, D)) * 0.02).astype(np.float32)
    Wk = (rng.standard_normal((512, D)) * 0.02).astype(np.float32)
    Wv = (rng.standard_normal((512, D)) * 0.02).astype(np.float32)
    Wp = (rng.standard_normal((D, D)) * 0.02).astype(np.float32)
    g = np.ones(H, np.float32)
    out = kernel(x, Wq, Wk, Wv, Wp, g)
    print("out", out.shape, out.dtype, float(np.abs(out).max()))



# revision 50
# speedup vs baseline: 1.1614x; 1.1614x over previous
"""Causal self-attention (GQA + RMS-norm + RoPE) Trainium2 Bass kernel.

Sharding over 8 NeuronCores: 2-way data parallel (batch) x 4-way head
parallel (one GQA group of 4 q-heads + 1 kv-head per core).  Each core
computes q/k/v projections for its group, flash-style causal attention
(scores kept transposed [k, q] so softmax sums ride the tensor engine),
and a partial output projection.  Host sums the 4 per-group partials per
batch.

All matmul operands are bf16 (fp32 accumulation in PSUM); softmax /
norm statistics are fp32.  RMS-norm bounds |scores| <= gain*sqrt(hd), so
softmax needs no max-subtraction.
"""

import numpy as np
import ml_dtypes

import concourse.bacc as bacc
import concourse.mybir as mybir
from concourse.tile import TileContext
from concourse.bass_utils import run_bass_kernel_spmd

BF16 = mybir.dt.bfloat16
F32 = mybir.dt.float32
F32R = mybir.dt.float32r
AF = mybir.ActivationFunctionType
bf = ml_dtypes.bfloat16

B, S, D = 2, 2048, 2048
H, HKV, HD = 16, 4, 128
RQ = H // HKV            # q heads per kv group (4)
NCORES = 8
NDT = D // 128           # 16 contraction tiles
NST = S // 512           # 4 query/sequence 512-tiles
NKT = S // 128           # 16 key 128-tiles
EPS = float(np.finfo(np.float32).eps)

_PROG_CACHE = {}


def _build_program(n_timing_iters=1, phases="full"):
    nc = bacc.Bacc("TRN2", debug=False, enable_asserts=False, num_devices=NCORES)

    xT_d = nc.dram_tensor("xT", [128, NDT, S], BF16, kind="ExternalInput")
    wqT_d = nc.dram_tensor("wqT", [128, NDT, RQ * HD], BF16, kind="ExternalInput")
    wkT_d = nc.dram_tensor("wkT", [128, NDT, HD], BF16, kind="ExternalInput")
    wvT_d = nc.dram_tensor("wvT", [128, NDT, HD], BF16, kind="ExternalInput")
    wpT_d = nc.dram_tensor("wpT", [NDT, 128, RQ * 128], BF16, kind="ExternalInput")
    cosF_d = nc.dram_tensor("cosF", [128, S], BF16, kind="ExternalInput")
    sinF_d = nc.dram_tensor("sinF", [128, S], BF16, kind="ExternalInput")
    cfs_d = nc.dram_tensor("cfs", [1, 641], F32R, kind="ExternalInput")
    onescol_f_d = nc.dram_tensor("onescol_f", [128, 1], F32R, kind="ExternalInput")
    onescol_b_d = nc.dram_tensor("onescol_b", [128, 1], BF16, kind="ExternalInput")
    idtr_d = nc.dram_tensor("idtr", [128, 256], BF16, kind="ExternalInput")
    outT_d = nc.dram_tensor("outT", [NDT, 128, S], BF16, kind="ExternalOutput")

    with TileContext(nc) as tc:
        with tc.tile_pool(name="res", bufs=1) as res, \
             tc.tile_pool(name="work", bufs=2) as wk, \
             tc.tile_pool(name="pwork", bufs=2, space="PSUM") as pw:

            # ---- resident tiles (allocated once) ----
            xT = res.tile([128, NDT, S], BF16)             # [d-part, dt, s]
            wqT = res.tile([128, NDT, RQ * HD], BF16)
            wkT = res.tile([128, NDT, HD], BF16)
            wvT = res.tile([128, NDT, HD], BF16)
            cosF = res.tile([128, S], BF16)
            sinF = res.tile([128, S], BF16)
            cfs = res.tile([1, 641], F32R)
            onescol_f = res.tile([128, 1], F32R)
            onescol_b = res.tile([128, 1], BF16)
            idtr = res.tile([128, 256], BF16)
            kT = res.tile([128, S], BF16)
            qT = [res.tile([128, S], BF16, name=f"qT{h}", tag=f"qT{h}")
                  for h in range(RQ)]
            yT = [res.tile([128, S], BF16, name=f"yT{h}", tag=f"yT{h}")
                  for h in range(RQ)]
            vTst = res.tile([128, S], BF16)                # v^T staging
            V_all = res.tile([128, S], BF16)               # v natural, kt-major

            eps_ap = cfs[0:1, 0:1].bitcast(F32)
            ones_row = cfs[0:1, 1:129]
            ident = idtr[:, 0:128]
            triu = idtr[:, 128:256]

            def body(_iv=None):
                # ---- batched resident loads ----
                nc.scalar.dma_start(wkT[:], wkT_d[:])
                nc.scalar.dma_start(wvT[:], wvT_d[:])
                for c in range(8):
                    eng = nc.sync if c % 2 == 0 else nc.scalar
                    eng.dma_start(xT[:, 2 * c:2 * c + 2, :],
                                  xT_d[:, 2 * c:2 * c + 2, :])
                nc.scalar.dma_start(wqT[:], wqT_d[:])
                nc.scalar.dma_start(cosF[:], cosF_d[:])
                nc.scalar.dma_start(sinF[:], sinF_d[:])
                nc.scalar.dma_start(cfs[:], cfs_d[:])
                nc.scalar.dma_start(onescol_f[:], onescol_f_d[:])
                nc.scalar.dma_start(onescol_b[:], onescol_b_d[:])
                nc.scalar.dma_start(idtr[:], idtr_d[:])
                nc.scalar.add_instruction(
                    mybir.InstLoadActFuncSet(
                        name=nc.get_next_instruction_name(),
                        ins=[], outs=[], act_func_set_id=6))

                if phases == "load":
                    return

                def proj_accum(wt_all, col_off, j, tag="big", tbufs=3):
                    """psum [128,512] = sum_d W[d].T @ xT[d, s-slice]"""
                    acc = pw.tile([128, 512], F32, name="acc", tag=tag,
                                  bufs=tbufs)
                    for dt in range(NDT):
                        lhsT = wt_all[:, dt, col_off:col_off + 128]
                        rhs = xT[:, dt, 512 * j:512 * j + 512]
                        nc.tensor.matmul(acc[:], lhsT, rhs,
                                         start=(dt == 0), stop=(dt == NDT - 1))
                    return acc

                def norm_rope_chain(acc, grow_slice, dest, j):
                    """RMS-norm + RoPE + scale; writes dest[:, 512j:+512] bf16.

                    Stats ride ACT/DVE/Pool; the only PE op is the ms
                    ones-matmul, emitted here (callers pipeline the call one
                    chain behind the mm chains to keep PE dense)."""
                    stage = wk.tile([128, 512], F32, name="stage", tag="stage", bufs=3)
                    nc.vector.tensor_copy(stage[:], acc[:])
                    swap = wk.tile([128, 512], F32, name="swap", tag="swap", bufs=3)
                    nc.sync.dma_start(swap[0:64, :], stage[64:128, :])
                    nc.sync.dma_start(swap[64:128, :], stage[0:64, :])
                    sq = wk.tile([128, 512], F32R, name="sq", tag="sq", bufs=2)
                    nc.vector.tensor_mul(sq[:], stage[:], stage[:])
                    ms = pw.tile([1, 512], F32, name="ms", tag="bcast", bufs=1)
                    nc.tensor.matmul(ms[:], onescol_f[:],
                                     sq[:], start=True, stop=True)
                    # rsqrt via ln+exp: keeps every ACT op in the single
                    # natural_log_exp_and_others table set (no set thrash
                    # against attention's Exp).
                    srt = wk.tile([1, 512], F32, name="srt", tag="srt", bufs=2)
                    nc.scalar.activation(srt[:], ms[:], AF.Ln,
                                         bias=eps_ap, scale=1.0 / HD)
                    rr = wk.tile([1, 512], F32, name="rr", tag="rr", bufs=2)
                    nc.scalar.activation(rr[:], srt[:], AF.Exp, scale=-0.5)
                    if grow_slice is not None:
                        # scale by gain/sqrt(hd) (per-head scalar)
                        rg = wk.tile([1, 512], F32, name="rg", tag="rg", bufs=2)
                        nc.vector.tensor_scalar_mul(rg[:], rr[:], grow_slice)
                    else:
                        rg = rr
                    Rb = wk.tile([128, 512], F32, name="Rb", tag="Rb", bufs=2)
                    nc.gpsimd.partition_broadcast(Rb[:], rg[0:1, :])
                    sl = slice(512 * j, 512 * j + 512)
                    nc.vector.tensor_mul(stage[:], stage[:], cosF[:, sl])
                    nc.vector.tensor_mul(swap[:], swap[:], sinF[:, sl])
                    nc.vector.tensor_add(stage[:], stage[:], swap[:])
                    nc.vector.tensor_mul(dest[:, sl], stage[:], Rb[:])

                # ---- projections: mm-chains pipelined one ahead of stats ----
                proj_jobs = []   # (kind, h, j); kind: 0=k, 1=v, 2=q
                for j in range(NST):
                    proj_jobs.append(("k", 0, j))
                for j in range(NST):
                    proj_jobs.append(("v", 0, j))

                def emit_proj_tail(kind, h, j, acc):
                    if kind == "k":
                        norm_rope_chain(acc, None, kT, j)
                    elif kind == "q":
                        grow_h = cfs[0:1, 129 + h:130 + h].bitcast(F32)
                        norm_rope_chain(acc, grow_h, qT[h], j)
                    else:
                        nc.vector.tensor_copy(vTst[:, 512 * j:512 * j + 512], acc[:])

                def run_proj_jobs(jobs, pending, tags=None, lag=1):
                    for idx, (kind, h, j) in enumerate(jobs):
                        tag, tbufs = ("big", 3) if tags is None else tags[idx]
                        if kind == "k":
                            acc = proj_accum(wkT, 0, j, tag, tbufs)
                        elif kind == "v":
                            acc = proj_accum(wvT, 0, j, tag, tbufs)
                        else:
                            acc = proj_accum(wqT, 128 * h, j, tag, tbufs)
                        pending.append((kind, h, j, acc))
                        if len(pending) > lag:
                            emit_proj_tail(*pending.pop(0))
                    return pending

                # first wave: 7-deep psum concurrency so the x-load DMA
                # wavefront keeps PE fed (every arriving x tile unlocks 7 mms)
                kv_tags = [("big", 3), ("big", 3), ("big", 3), ("acc", 2),
                           ("acc", 2), ("small", 2), ("small", 2), ("big", 3)]
                pending = run_proj_jobs(proj_jobs, [], tags=kv_tags, lag=6)

                # ---- v^T -> V transposes (PE) ----
                while pending:
                    emit_proj_tail(*pending.pop(0))
                for kt in range(NKT):
                    tp = pw.tile([128, 128], BF16, name="tp", tag="acc", bufs=2)
                    nc.tensor.transpose(tp[:], vTst[:, 128 * kt:128 * kt + 128],
                                        ident)
                    nc.scalar.copy(V_all[:, 128 * kt:128 * kt + 128], tp[:])

                if phases == "kv":
                    return

                def attention_block(h, j):
                    """Causal attention for queries [512j, 512j+512), head h.
                    d/PV matmuls trail the S-matmul/exp pipeline by 2 k-tiles
                    so PE never waits on ACT."""
                    nkt = 4 * j + 4
                    acc_y = pw.tile([128, 512], F32, name="acc_y", tag="acc",
                                    bufs=2)
                    acc_d = pw.tile([1, 512], F32, name="acc_d", tag="small",
                                    bufs=2)
                    lagged = []

                    def consume(kt, P, c0):
                        nc.tensor.matmul(acc_d[0:1, c0:512], onescol_b[:],
                                         P[:, c0:512],
                                         start=(kt == 0), stop=(kt == nkt - 1),
                                         skip_group_check=True)
                        nc.tensor.matmul(acc_y[:, c0:512],
                                         V_all[:, 128 * kt:128 * kt + 128],
                                         P[:, c0:512],
                                         start=(kt == 0), stop=(kt == nkt - 1),
                                         skip_group_check=True)

                    for kt in range(nkt):
                        c0 = max(0, 128 * (kt - 4 * j))
                        ps = pw.tile([128, 512], F32, name="ps", tag="big",
                                     bufs=3)
                        nc.tensor.matmul(
                            ps[:, c0:512],
                            kT[:, 128 * kt:128 * kt + 128],
                            qT[h][:, 512 * j + c0:512 * j + 512],
                            start=True, stop=True)
                        P = wk.tile([128, 512], BF16, name="P", tag="P", bufs=6)
                        nc.scalar.activation(P[:, c0:512], ps[:, c0:512], AF.Exp)
                        if kt >= 4 * j:
                            nc.vector.tensor_mul(P[:, c0:c0 + 128],
                                                 P[:, c0:c0 + 128], triu)
                        lagged.append((kt, P, c0))
                        if len(lagged) > 2:
                            consume(*lagged.pop(0))
                    while lagged:
                        consume(*lagged.pop(0))
                    rd = wk.tile([1, 512], F32, name="rd", tag="rd", bufs=2)
                    nc.vector.reciprocal(rd[:], acc_d[:])
                    Rd = wk.tile([128, 512], F32, name="Rd", tag="Rb", bufs=2)
                    nc.gpsimd.partition_broadcast(Rd[:], rd[0:1, :])
                    nc.vector.tensor_mul(yT[h][:, 512 * j:512 * j + 512],
                                         acc_y[:], Rd[:])

                # ---- per q-head: q(h+1) projections emitted ahead of
                # attention(h) so PE crosses head boundaries without gaps ----
                pending = run_proj_jobs([("q", 0, j) for j in range(NST)],
                                        pending)
                for h in range(RQ):
                    if h + 1 < RQ:
                        pending = run_proj_jobs(
                            [("q", h + 1, j) for j in range(NST)], pending)
                    while pending:
                        emit_proj_tail(*pending.pop(0))
                    if phases == "kvq":
                        continue
                    for j in range(NST):
                        attention_block(h, j)

                # ---- output projection (transposed: out^T[D, s]) ----
                if phases in ("kv", "kvq", "noout"):
                    return
                ptags = ["big", "acc", "bcast", "small"]
                pbufs = {"big": 3, "acc": 2, "bcast": 1, "small": 2}
                for dt in range(NDT):
                    wp = wk.tile([128, RQ * 128], BF16, name="wp", tag="wp",
                                 bufs=3)
                    nc.scalar.dma_start(wp[:], wpT_d[dt])
                    osb = wk.tile([128, S], BF16, name="osb", tag="osb")
                    for sjj in range(NST):
                        po = pw.tile([128, 512], F32, name=f"po{sjj}",
                                     tag=ptags[sjj], bufs=pbufs[ptags[sjj]])
                        for h in range(RQ):
                            nc.tensor.matmul(
                                po[:], wp[:, 128 * h:128 * h + 128],
                                yT[h][:, 512 * sjj:512 * sjj + 512],
                                start=(h == 0), stop=(h == RQ - 1))
                        if sjj % 2 == 0:
                            nc.vector.tensor_copy(
                                osb[:, 512 * sjj:512 * sjj + 512], po[:])
                        else:
                            nc.scalar.copy(
                                osb[:, 512 * sjj:512 * sjj + 512], po[:])
                    eng = nc.sync if dt % 2 == 0 else nc.scalar
                    eng.dma_start(outT_d[dt], osb[:])

            if n_timing_iters > 1:
                with tc.For_i(0, n_timing_iters, 1):
                    body()
            else:
                body()

    nc.compile()
    return nc


def _get_program(n_timing_iters=1, phases="full"):
    key = (n_timing_iters, phases)
    if key not in _PROG_CACHE:
        _PROG_CACHE[key] = _build_program(n_timing_iters, phases)
    return _PROG_CACHE[key]


def _host_inputs(x, Wq, Wk, Wv, Wproj, q_gain):
    """Build the 8 per-core input maps (host-side layout prep)."""
    half = HD // 2
    inv = 1.0 / (10000.0 ** (np.arange(0, HD, 2, dtype=np.float64) / HD))
    t = np.arange(S, dtype=np.float64)
    fr = np.outer(t, inv).astype(np.float32)          # [S, 64]
    cos = np.cos(fr).astype(np.float32)
    sin = np.sin(fr).astype(np.float32)
    cosF = np.concatenate([cos.T, cos.T], 0).astype(bf)          # [128, S]
    sinF = np.concatenate([sin.T, -sin.T], 0).astype(bf)

    onescol_f = np.ones((128, 1), np.float32)
    onescol_b = np.ones((128, 1), bf)
    ident = np.eye(128, dtype=np.float32)
    triu = (np.arange(128)[None, :] >= np.arange(128)[:, None]).astype(np.float32)
    idtr = np.concatenate([ident, triu], 1).astype(bf)

    def pmajor(wT, width):
        return np.ascontiguousarray(
            wT.reshape(NDT, 128, width).transpose(1, 0, 2))

    xT = [pmajor(np.ascontiguousarray(x[b].T).astype(bf), S) for b in range(B)]

    in_maps = []
    for c in range(NCORES):
        b, g = divmod(c, HKV) if NCORES == B * HKV else (None, None)
        # core layout: c = b * 4 + g
        b, g = c // HKV, c % HKV
        wq = np.ascontiguousarray(Wq[512 * g:512 * (g + 1)].T).astype(bf)
        wk_ = np.ascontiguousarray(Wk[128 * g:128 * (g + 1)].T).astype(bf)
        wv = np.ascontiguousarray(Wv[128 * g:128 * (g + 1)].T).astype(bf)
        wpT = np.ascontiguousarray(Wproj[:, 512 * g:512 * (g + 1)].T)  # [512, 2048]
        # [dt][c-part 128, (h, m) 512]: wpT2[dt, c, 128h+m] = Wp[128dt+m, 512g+128h+c]
        wpT = np.ascontiguousarray(
            wpT.reshape(RQ, 128, NDT, 128).transpose(2, 1, 0, 3).reshape(
                NDT, 128, RQ * 128)).astype(bf)
        cfsv = np.zeros((1, 641), np.float32)
        cfsv[0, 0] = EPS
        cfsv[0, 1:129] = 1.0
        gv = (q_gain[RQ * g: RQ * (g + 1)].astype(np.float64)
              / np.sqrt(HD)).astype(np.float32)
        cfsv[0, 129:133] = gv
        in_maps.append({
            "xT": xT[b],
            "wqT": pmajor(wq, RQ * HD),
            "wkT": pmajor(wk_, HD),
            "wvT": pmajor(wv, HD),
            "wpT": wpT,
            "cosF": cosF, "sinF": sinF, "cfs": cfsv,
            "onescol_f": onescol_f, "onescol_b": onescol_b, "idtr": idtr,
        })
    return in_maps


def kernel(x, Wq, Wk, Wv, Wproj, q_gain, _n_timing_iters=1, _return_raw=False):
    x = np.asarray(x, np.float32)
    in_maps = _host_inputs(np.asarray(x, np.float32),
                           np.asarray(Wq, np.float32),
                           np.asarray(Wk, np.float32),
                           np.asarray(Wv, np.float32),
                           np.asarray(Wproj, np.float32),
                           np.asarray(q_gain, np.float32))
    nc = _get_program(_n_timing_iters)
    res = run_bass_kernel_spmd(nc, in_maps, core_ids=list(range(NCORES)),
                               trace=False)
    if _return_raw:
        return res
    out = np.zeros((B, S, D), np.float32)
    for c in range(NCORES):
        b = c // HKV
        outT = res.results[c]["outT"].reshape(D, S).astype(np.float32)
        out[b] += outT.T
    return out


if __name__ == "__main__":
    rng = np.random.default_rng(0)
    x = rng.standard_normal((B, S, D)).astype(np.float32)
    Wq = (rng.standard_normal((D, D)) * 0.02).astype(np.float32)
    Wk = (rng.standard_normal((512, D)) * 0.02).astype(np.float32)
    Wv = (rng.standard_normal((512, D)) * 0.02).astype(np.float32)
    Wp = (rng.standard_normal((D, D)) * 0.02).astype(np.float32)
    g = np.ones(H, np.float32)
    out = kernel(x, Wq, Wk, Wv, Wp, g)
    print("out", out.shape, out.dtype, float(np.abs(out).max()))

